# revision 28
# baseline (speedup 1.0000x reference)
"""Trainium2 Bass kernel: causal multi-head attention with RoPE.

Problem: B=2, S=2048, D=1024, H=16 heads, hd=64, fp32.
Sharding: 4-way head-tensor-parallel x 2-way batch-data-parallel over 8 cores.
Each core handles one batch element and 4 heads (256 of the 1024 model dims),
computes its partial contribution to the output projection, and the host sums
the 4 partials per batch element.

Per-core pipeline (bf16 matmul operands, fp32 PSUM accumulation):
  - The PE stream is the budget: every matmul occupies the array for its
    moving-column count EXCEPT adjacent matmuls on disjoint row groups,
    which co-stream. The scores matmuls (K=64 per head) are emitted
    e-outer/z-inner so the (row 0-63, row 64-127) head pair is adjacent
    in issue order and overlaps; emitted z-outer they head-of-line block
    and fully serialize (~14.5us wasted).
  - Projection work is scheduled as round-granular FILLERS inside the
    attention windows (which are ACT-exp-bound): att(qc) consumes the
    q/k projection groups of chunk qc+1 between its kp rounds, and
    att(3) consumes the chunk-3 v-projections (legal: they are only
    read by its last two rounds). This keeps the PE fed and the HAM
    clock warm through the whole kernel.
  - RoPE groups alternate between DVE and GpSimd (both read/write SBUF
    only) so the rope chain is never the phase bottleneck; PSUM->SBUF
    staging (projection casts, v casts, O/den, y) stays on DVE; ACT
    does nothing but exp.
  - scores^T = k_rot^T.T @ q_rot^T per (head-pair, q-chunk, k-block-pair);
    causal block skip plus column-suffix restriction on the 4 diagonal
    blocks. The in-block causal triangle is a PE-side additive bias (-240
    above the diagonal via a constant triangular stationary times
    identity) so masking never crosses engines.
  - exp on ACT (scale=1/8 fused) over alive column ranges only; pt in
    bf16; the first diagonal round fuses both e-blocks into one exp
    (the dead gap holds bounded stale PSUM and is never read).
  - P^T @ v via matmul with a ones-column appended to v so the softmax
    denominator accumulates free in PSUM row 64; the stationary is
    padded to 128 cols (overreading into the next head / zero pad) so
    FWL stays enabled; PV lags the scores by two rounds.
  - O^T+den staged out of PSUM in one [65,512] copy per head (frees the
    PSUM bank fast); 1/den via DVE reciprocal_approx_fast; per-head
    partition_broadcast + normalize multiply on GpSimd into bf16.
  - y chunk = O_norm @ Wo_slice.T per q-chunk, emitted inside the next
    chunk's kp stream; the last chunk's projection is split per
    head-pair (hp0 inside hp1's kp stream, hp1 added at the end) so the
    tail never leaves the PE idle-then-bursty.
"""
import numpy as np
import ml_dtypes
from contextlib import ExitStack

import concourse.bass as bass
import concourse.tile as tile
from concourse import bacc, mybir
from concourse.bass_utils import run_bass_kernel_spmd

F32 = mybir.dt.float32
BF16 = mybir.dt.bfloat16

B, S, D, H, HD = 2, 2048, 1024, 16, 64
NCORES = 8
TPG = 4            # head-TP degree (groups of 4 heads)
LH = H // TPG      # 4 local heads per core
LD = LH * HD       # 256 local dims
ROPE_BASE = 10000.0
QC = 512           # q chunk (matmul moving dim)
NQC = S // QC      # 4
NST = S // 128     # 16 s tiles
NDT = D // 128     # 8 d tiles
NWARM = 6          # scratch matmuls to pre-warm the PE HAM clock

Exp = mybir.ActivationFunctionType.Exp

_NC_CACHE = None


def _build():
    nc = bacc.Bacc("TRN2", target_bir_lowering=False, debug=False,
                   enable_asserts=True, num_devices=NCORES)

    xbf = nc.dram_tensor("xbf", [NQC, NDT, 128, QC], BF16,
                         kind="ExternalInput").ap()
    wqt = nc.dram_tensor("wqt", [128, 2048], BF16, kind="ExternalInput").ap()
    wkt = nc.dram_tensor("wkt", [128, 2048], BF16, kind="ExternalInput").ap()
    wvt = nc.dram_tensor("wvt", [128, 2048], BF16, kind="ExternalInput").ap()
    wot = nc.dram_tensor("wot", [2, 128, D], BF16, kind="ExternalInput").ap()
    cosd = nc.dram_tensor("cosd", [128, S], BF16, kind="ExternalInput").ap()
    sind = nc.dram_tensor("sind", [128, S], BF16, kind="ExternalInput").ap()
    trib = nc.dram_tensor("trib", [128, 128], BF16,
                          kind="ExternalInput").ap()
    identd = nc.dram_tensor("identd", [128, 128], BF16,
                            kind="ExternalInput").ap()
    y = nc.dram_tensor("y", [S, D], F32, kind="ExternalOutput").ap()

    with tile.TileContext(nc) as tc, ExitStack() as octx:
        # ---- pools ----
        pers = octx.enter_context(tc.tile_pool(name="pers", bufs=1))
        qkp = octx.enter_context(tc.tile_pool(name="qkp", bufs=1))
        vp = octx.enter_context(tc.tile_pool(name="vp", bufs=1))
        xtp = octx.enter_context(tc.tile_pool(name="xtp", bufs=1))
        s1p = octx.enter_context(tc.tile_pool(name="s1p", bufs=1))
        s2p = octx.enter_context(tc.tile_pool(name="s2p", bufs=1))
        # PSUM: gen (proj/out-proj/warmup) 2 banks + scores 4 + PV 2 = 8
        pgen = octx.enter_context(tc.tile_pool(name="pgen", bufs=2,
                                               space="PSUM"))
        pss = octx.enter_context(tc.tile_pool(name="pss", bufs=2,
                                              space="PSUM"))
        pso = octx.enter_context(tc.tile_pool(name="pso", bufs=2,
                                              space="PSUM"))

        # q/k rotated, per head-pair tile: rows = [hA: y1(32)|y2(32) | hB:..]
        qrot = [qkp.tile([128, S], BF16, tag=f"qrot{j}", name=f"qrot{j}")
                for j in range(2)]
        krot = [qkp.tile([128, S], BF16, tag=f"krot{j}", name=f"krot{j}")
                for j in range(2)]
        # v natural with per-head ones column: cols lh*65..lh*65+64 = v head
        # lh, col lh*65+64 = 1.0. Padded to 324 cols so the PV stationary
        # is always a 128-col slice (keeps FWL enabled).
        vsb = [vp.tile([128, 324], BF16, tag=f"v{st}", name=f"v{st}")
               for st in range(NST)]

        # ---- input DMAs ----
        # x: two big strided DMAs per chunk on the sync queue (big
        # transfers get full HBM bandwidth)
        xt_all = xtp.tile([128, NDT * S], BF16, tag="xt", name="xt")
        xt = [xt_all[:, dt * S:(dt + 1) * S] for dt in range(NDT)]
        xt_3d = xt_all.rearrange("p (d c) -> p d c", c=S)
        for sc in range(NQC):
            c0, c1 = sc * QC, (sc + 1) * QC
            for h in range(2):
                d0, d1 = h * 4, h * 4 + 4
                nc.sync.dma_start(
                    xt_3d[:, d0:d1, c0:c1],
                    xbf[sc][d0:d1].rearrange("d p c -> p d c"))

        # weights split across the gpsimd/scalar queues, ordered by need
        wq = pers.tile([128, 2048], BF16, tag="wq", name="wq")
        nc.gpsimd.dma_start(wq[:, 0:1024], wqt[:, 0:1024])
        nc.scalar.dma_start(wq[:, 1024:2048], wqt[:, 1024:2048])
        wk = pers.tile([128, 2048], BF16, tag="wk", name="wk")
        nc.gpsimd.dma_start(wk[:, 0:1024], wkt[:, 0:1024])
        nc.scalar.dma_start(wk[:, 1024:2048], wkt[:, 1024:2048])
        cos_sb = pers.tile([128, S], BF16, tag="cos")
        sin_sb = pers.tile([128, S], BF16, tag="sin")
        nc.scalar.dma_start(cos_sb[:, 0:1024], cosd[:, 0:1024])
        nc.scalar.dma_start(sin_sb[:, 0:1024], sind[:, 0:1024])
        wv = pers.tile([128, 2048], BF16, tag="wv", name="wv")
        nc.gpsimd.dma_start(wv[:], wvt)
        tri = pers.tile([128, 128], BF16, tag="tri")
        nc.scalar.dma_start(tri[:], trib)
        ident = pers.tile([128, 128], BF16, tag="ident")
        nc.scalar.dma_start(ident[:], identd)
        nc.scalar.dma_start(cos_sb[:, 1024:2048], cosd[:, 1024:2048])
        nc.scalar.dma_start(sin_sb[:, 1024:2048], sind[:, 1024:2048])
        wo_sb = []
        for hp in range(2):
            w = pers.tile([128, D], BF16, tag=f"wo{hp}", name=f"wo{hp}")
            nc.gpsimd.dma_start(w[:], wot[hp])
            wo_sb.append(w)

        # scratch operand for HAM warmup matmuls
        scr = pers.tile([128, QC], BF16, tag="scr")
        nc.vector.memset(scr[:], 0.001)
        for st in range(NST):
            vdst = vsb[st][:, 0:260].rearrange("p (h c) -> p h c",
                                               c=65)[:, :, 64:65]
            nc.vector.memset(vdst, 1.0)
            nc.vector.memset(vsb[st][:, 260:324], 0.0)

        # pre-clear the two scores PSUM buffers: the fused diagonal exp
        # reads a dead gap the matmuls never wrote
        ps_init = [pss.tile([128, 1024], F32, tag="ps", name="ps_")
                   for _ in range(2)]
        for t in ps_init:
            nc.vector.memset(t[:], 0.0)

        def warm_mm():
            wt = pgen.tile([128, QC], F32, tag="gen", name="gen")
            nc.tensor.matmul(wt[:], scr[:, 0:128], scr[:],
                             start=True, stop=True)

        for _ in range(NWARM):
            warm_mm()

        # ---- projection groups ----
        # raw q/k projections staged to SBUF via ACT (idle during the
        # projection phases); RoPE runs as separate half-S DVE passes so
        # its per-op overhead is halved and it never gates the PE's
        # PSUM-pool rotation
        xq = [s1p.tile([128, S], BF16, tag=f"xq{i}", name=f"xq{i}")
              for i in range(4)]

        def emit_QK(sc, wi, jt, warm=False):
            c0, c1 = sc * QC, (sc + 1) * QC
            w = wq if wi == 0 else wk
            if warm:
                warm_mm()
            pq = pgen.tile([128, QC], F32, tag="gen", name="gen")
            for dt in range(NDT):
                nc.tensor.matmul(
                    pq[:],
                    w[:, dt * 256 + jt * 128:dt * 256 + jt * 128 + 128],
                    xt[dt][:, c0:c1],
                    start=(dt == 0), stop=(dt == NDT - 1))
            nc.scalar.copy(xq[wi * 2 + jt][:, c0:c1], pq[:])

        def emit_rope(wi, jt, h):
            c0, c1 = h * 1024, (h + 1) * 1024
            rot = qrot if wi == 0 else krot
            xb = xq[wi * 2 + jt]
            t1 = s1p.tile([128, 1024], BF16, tag="t1", bufs=2, name="t1")
            nc.vector.tensor_mul(t1[:], xb[:, c0:c1], cos_sb[:, c0:c1])
            # t2[p] = x[swap32(p)] * sneg[swap32(p)]: swap32 flips adjacent
            # 32-row blocks; sneg carries -sin on the x2 rows, so
            # rot = t1 + t2 is the full rotation. (in-operands must share
            # a base partition; only the out base may shift.)
            t2 = s1p.tile([128, 1024], BF16, tag="t2", bufs=2, name="t2")
            for blk in range(4):
                a, b_ = blk * 32, (blk ^ 1) * 32
                nc.vector.tensor_mul(t2[a:a + 32, :],
                                     xb[b_:b_ + 32, c0:c1],
                                     sin_sb[b_:b_ + 32, c0:c1])
            nc.vector.tensor_add(rot[jt][:, c0:c1], t1[:], t2[:])

        def emit_V(st, warm=False):
            if warm:
                warm_mm()
            pv = pgen.tile([128, QC], F32, tag="gen", name="gen")
            for dt in range(NDT):
                nc.tensor.matmul(
                    pv[:, 0:256],
                    xt[dt][:, st * 128:(st + 1) * 128],
                    wv[:, dt * 256:(dt + 1) * 256],
                    start=(dt == 0), stop=(dt == NDT - 1))
            dst = vsb[st][:, 0:260].rearrange("p (h c) -> p h c",
                                              c=65)[:, :, 0:64]
            src = pv[:, 0:256].rearrange("p (h c) -> p h c", c=64)
            nc.scalar.copy(dst, src)

        # ---- output projection ----
        pend_y = [None]

        def emit_y(qc, otn):
            for j in range(4):
                st = 4 * qc + j
                ysb = s2p.tile([128, D], F32, tag="ysb", bufs=3,
                               name="ysb")
                for mc in range(2):
                    py = pgen.tile([128, QC], F32, tag="gen", name="gen")
                    for hp in range(2):
                        nc.tensor.matmul(
                            py[:],
                            otn[hp][:, j * 128:(j + 1) * 128],
                            wo_sb[hp][:, mc * 512:(mc + 1) * 512],
                            start=(hp == 0), stop=(hp == 1))
                    nc.vector.tensor_copy(
                        ysb[:, mc * 512:(mc + 1) * 512], py[:])
                    q_ = nc.sync if mc == 0 else nc.gpsimd
                    q_.dma_start(
                        y[st * 128:(st + 1) * 128,
                          mc * 512:(mc + 1) * 512],
                        ysb[:, mc * 512:(mc + 1) * 512])

        # qc=3 output projection, split per head-pair so the PE isn't
        # idle-then-bursty after the last ACT-bound attention rounds
        y3sb = [s2p.tile([128, D], F32, tag="y3", bufs=4, name="y3")
                for _ in range(4)]

        def emit_y3_part0(otn0):
            for j in range(4):
                for mc in range(2):
                    py = pgen.tile([128, QC], F32, tag="gen", name="gen")
                    nc.tensor.matmul(
                        py[:], otn0[:, j * 128:(j + 1) * 128],
                        wo_sb[0][:, mc * 512:(mc + 1) * 512],
                        start=True, stop=True)
                    nc.vector.tensor_copy(
                        y3sb[j][:, mc * 512:(mc + 1) * 512], py[:])

        def emit_y3_final(otn1):
            for j in range(4):
                st = 12 + j
                for mc in range(2):
                    py = pgen.tile([128, QC], F32, tag="gen", name="gen")
                    nc.tensor.matmul(
                        py[:], otn1[:, j * 128:(j + 1) * 128],
                        wo_sb[1][:, mc * 512:(mc + 1) * 512],
                        start=True, stop=True)
                    nc.vector.tensor_add(
                        y3sb[j][:, mc * 512:(mc + 1) * 512],
                        y3sb[j][:, mc * 512:(mc + 1) * 512], py[:])
                    q_ = nc.sync if mc == 0 else nc.gpsimd
                    q_.dma_start(
                        y[st * 128:(st + 1) * 128,
                          mc * 512:(mc + 1) * 512],
                        y3sb[j][:, mc * 512:(mc + 1) * 512])

        # ---- attention window for one q-chunk ----
        def emit_att(qc, fillers=()):
            fillers = list(fillers)
            fi = [0]

            def take_filler():
                if fi[0] < len(fillers):
                    fillers[fi[0]]()
                    fi[0] += 1

            npair = 2 * qc + 2
            otn = [s2p.tile([128, QC], BF16, tag=f"otn{hp}", bufs=2,
                            name=f"otn{hp}") for hp in range(2)]
            for hp in range(2):          # head pair = (2hp, 2hp+1)
                po = [pso.tile([128, QC], F32, tag="po", name="po")
                      for _ in range(2)]

                def emit_pv(kp, pts, mlist, last, hp=hp, po=po):
                    for z in range(2):
                        lh = 2 * hp + z
                        for e in range(2):
                            kb = 2 * kp + e
                            m = mlist[e]
                            nc.tensor.matmul(
                                po[z][0:128, m:QC],
                                vsb[kb][:, lh * 65:lh * 65 + 128],
                                pts[z][:, e * QC + m:(e + 1) * QC],
                                start=(kb == 0),
                                stop=(last and e == 1))

                pend = []
                for ki in range(npair):
                    kp = ki
                    diag = kp >= npair - 2
                    mlist = []
                    for e in range(2):
                        kb = 2 * kp + e
                        r = kb - 4 * qc
                        mlist.append(128 * r if diag else 0)
                    ps_pair = [pss.tile([128, 1024], F32, tag="ps",
                                        name="ps_") for _ in range(2)]
                    # e-outer / z-inner: the two z matmuls hit disjoint
                    # PE row groups and are adjacent in issue order, so
                    # they co-stream (z-outer order head-of-line blocks
                    # on the same-row-group e sibling and serializes)
                    for e in range(2):
                        kb = 2 * kp + e
                        m = mlist[e]
                        for z in range(2):
                            r0 = 64 * z
                            nc.tensor.matmul(
                                ps_pair[z][:, e * QC + m:(e + 1) * QC],
                                krot[hp][r0:r0 + 64,
                                         kb * 128:(kb + 1) * 128],
                                qrot[hp][r0:r0 + 64,
                                         qc * QC + m:(qc + 1) * QC],
                                start=True, stop=not diag,
                                tile_position=(r0, 0))
                    if diag:
                        # causal triangle as a PE-side additive bias
                        # (-240 above the diagonal) on the first 128
                        # alive columns
                        for e in range(2):
                            m = mlist[e]
                            for z in range(2):
                                nc.tensor.matmul(
                                    ps_pair[z][:,
                                               e * QC + m:e * QC + m + 128],
                                    tri[:], ident[:],
                                    start=False, stop=True,
                                    skip_group_check=True)
                    pts = []
                    for z in range(2):
                        pt = s2p.tile([128, 1024], BF16, tag="pt",
                                      bufs=6, name="pt")
                        if not diag:
                            nc.scalar.activation(pt[:], ps_pair[z][:],
                                                 Exp, scale=0.125)
                        elif mlist[0] == 0:
                            # first diagonal round: one exp across both
                            # e-blocks; the dead gap cols hold bounded
                            # stale PSUM and are never read by PV
                            nc.scalar.activation(pt[:], ps_pair[z][:],
                                                 Exp, scale=0.125)
                        else:
                            for e in range(2):
                                m = mlist[e]
                                nc.scalar.activation(
                                    pt[:, e * QC + m:(e + 1) * QC],
                                    ps_pair[z][:, e * QC + m:(e + 1) * QC],
                                    Exp, scale=0.125)
                        pts.append(pt)
                    # software-pipeline: PV lags two rounds behind the
                    # scores so the PE never waits on exp latency
                    pend.append((kp, pts, mlist, ki == npair - 1))
                    if len(pend) > 2:
                        emit_pv(*pend.pop(0))
                    if ki == 1 and hp == 1 and pend_y[0] is not None:
                        emit_y(*pend_y[0])
                        pend_y[0] = None
                    if qc == 3 and hp == 1 and ki == 4:
                        emit_y3_part0(otn[0])
                    if ki < npair - 1:
                        take_filler()
                for args in pend:
                    emit_pv(*args)

                # stage O^T+den out of PSUM in one copy per z (frees po
                # for the next head pair), then 1/den -> broadcast ->
                # normalize; broadcast+multiply live on GpSimd
                otu2 = []
                dsb2 = []
                for z in range(2):
                    ot = s2p.tile([64, QC], F32, tag="otu", bufs=4,
                                  name="otu")
                    nc.vector.tensor_copy(ot[:], po[z][0:64, :])
                    otu2.append(ot)
                    ds = s2p.tile([1, QC], F32, tag="dsb", bufs=4,
                                  name="dsb")
                    nc.vector.tensor_copy(ds[:], po[z][64:65, :])
                    dsb2.append(ds)
                for z in range(2):
                    r = s2p.tile([1, QC], F32, tag="rsb", bufs=4,
                                 name="rsb")
                    nc.vector.reciprocal_approx_fast(
                        out=r[:], in_=dsb2[z][:])
                    pbs = s2p.tile([64, QC], F32, tag="pbs", bufs=4,
                                   name="pbs")
                    nc.gpsimd.partition_broadcast(pbs[:], r[:])
                    nc.vector.tensor_mul(otn[hp][64 * z:64 * z + 64, :],
                                         otu2[z][:], pbs[:])
            pend_y[0] = (qc, otn)
            # leftover fillers (window had fewer slots than fillers)
            while fi[0] < len(fillers):
                fillers[fi[0]]()
                fi[0] += 1

        # ---- schedule: interleave projection chunks between the ----
        # ---- ACT-bound attention windows; dependency-free warm  ----
        # ---- matmuls keep HAM at full clock through the         ----
        # ---- exp-bound rounds                                   ----
        def emit_P(sc, warm=False):
            for wi in range(2):
                for jt in range(2):
                    emit_QK(sc, wi, jt, warm=warm)
            for st in range(4 * sc, 4 * sc + 4):
                emit_V(st, warm=warm)

        emit_P(0, warm=True)
        emit_P(1, warm=True)
        for wi in range(2):
            for jt in range(2):
                emit_rope(wi, jt, 0)
        emit_att(0, fillers=[warm_mm] * 2)
        emit_P(2)
        emit_att(1, fillers=[warm_mm] * 6)
        emit_P(3)
        for wi in range(2):
            for jt in range(2):
                emit_rope(wi, jt, 1)
        emit_att(2, fillers=[warm_mm] * 10)
        emit_att(3, fillers=[warm_mm] * 12)
        emit_y3_final(pend_y[0][1][1])

    nc.compile()
    return nc


def _get_nc():
    global _NC_CACHE
    if _NC_CACHE is None:
        _NC_CACHE = _build()
    return _NC_CACHE


def _host_prep(x, Wq, Wk, Wv, Wo):
    """Build the 8 per-core input maps."""
    bf = ml_dtypes.bfloat16
    x = np.asarray(x, dtype=np.float32)
    Wq, Wk, Wv, Wo = (np.asarray(w, dtype=np.float32) for w in (Wq, Wk, Wv, Wo))

    def tile128(wt):  # [1024, 256] -> [128, 2048] with d-tiles along free dim
        return np.ascontiguousarray(
            wt.reshape(NDT, 128, LD).transpose(1, 0, 2).reshape(
                128, NDT * LD).astype(bf))

    def perm_qk(W, g):
        # rows per jt tile: [h0 evens(32) | h0 odds(32) | h1 evens | h1 odds]
        blocks = []
        for lh in range(LH):
            gh = g * LH + lh
            O = W[gh * HD:(gh + 1) * HD]          # [64, 1024]
            blocks.append(O[0::2])
            blocks.append(O[1::2])
        Wp = np.concatenate(blocks, axis=0)       # [256, 1024]
        return tile128(Wp.T)

    t = np.arange(32, dtype=np.float64)
    theta = 1.0 / (ROPE_BASE ** (2.0 * t / HD))
    ang = np.arange(S, dtype=np.float64)[:, None] * theta[None, :]  # [S, 32]
    c32 = np.cos(ang).T.astype(np.float32)        # [32, S]
    s32 = np.sin(ang).T.astype(np.float32)
    cosd = np.ascontiguousarray(np.tile(c32, (4, 1)).astype(bf))  # [128, S]
    sneg = np.tile(s32, (4, 1))
    sneg[32:64] *= -1.0   # x2 rows carry -sin so rot = x*cos + swap(x*sneg)
    sneg[96:128] *= -1.0
    sind = np.ascontiguousarray(sneg.astype(bf))

    k_ = np.arange(128)[:, None]
    p_ = np.arange(128)[None, :]
    # additive causal bias, laid out [k, p]: out[p, j] += trib[j, p]
    trib = np.ascontiguousarray((-240.0 * (p_ > k_)).astype(bf))
    identd = np.ascontiguousarray(np.eye(128).astype(bf))

    per_b = []
    for b in range(B):
        # pre-transposed, chunked [sc, dt, 128 (d), QC (s)]
        xb = x[b].astype(bf)                       # [S, D]
        xb = xb.reshape(NQC, QC, NDT, 128).transpose(0, 2, 3, 1)
        per_b.append(np.ascontiguousarray(xb))

    per_g = []
    for g in range(TPG):
        wq = perm_qk(Wq, g)
        wk = perm_qk(Wk, g)
        wvt_ = tile128(Wv[g * LD:(g + 1) * LD].T)
        wot_ = np.ascontiguousarray(
            Wo[:, g * LD:(g + 1) * LD].T.reshape(2, 128, D).astype(bf))
        per_g.append((wq, wk, wvt_, wot_))

    in_maps = []
    for core in range(NCORES):
        b, g = divmod(core, TPG)
        wq, wk, wvt_, wot_ = per_g[g]
        in_maps.append({
            "xbf": per_b[b],
            "wqt": wq, "wkt": wk, "wvt": wvt_, "wot": wot_,
            "cosd": cosd, "sind": sind, "trib": trib, "identd": identd,
        })
    return in_maps


def run(inputs, trace=False):
    """Run on all 8 cores; returns (y_full, BassKernelResults)."""
    x = inputs["x"]
    in_maps = _host_prep(x, inputs["Wq"], inputs["Wk"], inputs["Wv"],
                         inputs["Wo"])
    nc = _get_nc()
    kw = {}
    if trace:
        kw = dict(trace=True, trace_cores=[0])
    res = run_bass_kernel_spmd(nc, in_maps, core_ids=list(range(NCORES)), **kw)
    y = np.zeros((B, S, D), dtype=np.float32)
    for c in range(NCORES):
        y[c // TPG] += res.results[c]["y"]
    return y, res


def kernel(x, Wq, Wk, Wv, Wo, n_heads):
    assert int(n_heads) == H
    y, _ = run({"x": x, "Wq": Wq, "Wk": Wk, "Wv": Wv, "Wo": Wo})
    return y


# revision 31
# speedup vs baseline: 1.1816x; 1.1816x over previous
"""Trainium2 Bass kernel: causal multi-head attention with RoPE.

Problem: B=2, S=2048, D=1024, H=16 heads, hd=64, fp32.
Sharding: 4-way head-tensor-parallel x 2-way batch-data-parallel over 8 cores.
Each core handles one batch element and 4 heads (256 of the 1024 model dims),
computes its partial contribution to the output projection, and the host sums
the 4 partials per batch element.

Per-core pipeline (bf16 matmul operands, fp32 PSUM accumulation):
  - The PE stream is the budget: every matmul occupies the array for its
    moving-column count EXCEPT adjacent matmuls on disjoint row groups,
    which co-stream. The scores matmuls (K=64 per head) are emitted
    e-outer/z-inner so the (row 0-63, row 64-127) head pair is adjacent
    in issue order and overlaps; emitted z-outer they head-of-line block
    and fully serialize (~14.5us wasted).
  - Projection work is scheduled as round-granular FILLERS inside the
    attention windows (which are ACT-exp-bound): att(qc) consumes the
    q/k projection groups of chunk qc+1 between its kp rounds, and
    att(3) consumes the chunk-3 v-projections (legal: they are only
    read by its last two rounds). This keeps the PE fed and the HAM
    clock warm through the whole kernel.
  - RoPE groups alternate between DVE and GpSimd (both read/write SBUF
    only) so the rope chain is never the phase bottleneck; PSUM->SBUF
    staging (projection casts, v casts, O/den, y) stays on DVE; ACT
    does nothing but exp.
  - scores^T = k_rot^T.T @ q_rot^T per (head-pair, q-chunk, k-block-pair);
    causal block skip plus column-suffix restriction on the 4 diagonal
    blocks. The in-block causal triangle is a PE-side additive bias (-240
    above the diagonal via a constant triangular stationary times
    identity) so masking never crosses engines.
  - exp on ACT (scale=1/8 fused) over alive column ranges only; pt in
    bf16; the first diagonal round fuses both e-blocks into one exp
    (the dead gap holds bounded stale PSUM and is never read).
  - P^T @ v via matmul with a ones-column appended to v so the softmax
    denominator accumulates free in PSUM row 64; the stationary is
    padded to 128 cols (overreading into the next head / zero pad) so
    FWL stays enabled; PV lags the scores by two rounds.
  - O^T+den staged out of PSUM in one [65,512] copy per head (frees the
    PSUM bank fast); 1/den via DVE reciprocal_approx_fast; per-head
    partition_broadcast + normalize multiply on GpSimd into bf16.
  - y chunk = O_norm @ Wo_slice.T per q-chunk, emitted inside the next
    chunk's kp stream; the last chunk's projection is split per
    head-pair (hp0 inside hp1's kp stream, hp1 added at the end) so the
    tail never leaves the PE idle-then-bursty.
"""
import numpy as np
import ml_dtypes
from contextlib import ExitStack

import concourse.bass as bass
import concourse.tile as tile
from concourse import bacc, mybir
from concourse.bass_utils import run_bass_kernel_spmd

F32 = mybir.dt.float32
BF16 = mybir.dt.bfloat16

B, S, D, H, HD = 2, 2048, 1024, 16, 64
NCORES = 8
TPG = 4            # head-TP degree (groups of 4 heads)
LH = H // TPG      # 4 local heads per core
LD = LH * HD       # 256 local dims
ROPE_BASE = 10000.0
QC = 512           # q chunk (matmul moving dim)
NQC = S // QC      # 4
NST = S // 128     # 16 s tiles
NDT = D // 128     # 8 d tiles
NWARM = 6          # scratch matmuls to pre-warm the PE HAM clock

Exp = mybir.ActivationFunctionType.Exp

_NC_CACHE = None


def _build():
    nc = bacc.Bacc("TRN2", target_bir_lowering=False, debug=False,
                   enable_asserts=True, num_devices=NCORES)

    xbf = nc.dram_tensor("xbf", [NQC, NDT, 128, QC], BF16,
                         kind="ExternalInput").ap()
    wqt = nc.dram_tensor("wqt", [128, 2048], BF16, kind="ExternalInput").ap()
    wkt = nc.dram_tensor("wkt", [128, 2048], BF16, kind="ExternalInput").ap()
    wvt = nc.dram_tensor("wvt", [128, 2048], BF16, kind="ExternalInput").ap()
    wot = nc.dram_tensor("wot", [2, 128, D], BF16, kind="ExternalInput").ap()
    cosd = nc.dram_tensor("cosd", [128, S], BF16, kind="ExternalInput").ap()
    sind = nc.dram_tensor("sind", [128, S], BF16, kind="ExternalInput").ap()
    trib = nc.dram_tensor("trib", [128, 128], BF16,
                          kind="ExternalInput").ap()
    identd = nc.dram_tensor("identd", [128, 128], BF16,
                            kind="ExternalInput").ap()
    y = nc.dram_tensor("y", [S, D], F32, kind="ExternalOutput").ap()

    with tile.TileContext(nc) as tc, ExitStack() as octx:
        # ---- pools ----
        pers = octx.enter_context(tc.tile_pool(name="pers", bufs=1))
        qkp = octx.enter_context(tc.tile_pool(name="qkp", bufs=1))
        vp = octx.enter_context(tc.tile_pool(name="vp", bufs=1))
        xtp = octx.enter_context(tc.tile_pool(name="xtp", bufs=1))
        s1p = octx.enter_context(tc.tile_pool(name="s1p", bufs=1))
        s2p = octx.enter_context(tc.tile_pool(name="s2p", bufs=1))
        # PSUM: gen (proj/out-proj/warmup) 2 banks + scores 4 + PV 2 = 8
        pgen = octx.enter_context(tc.tile_pool(name="pgen", bufs=2,
                                               space="PSUM"))
        pss = octx.enter_context(tc.tile_pool(name="pss", bufs=2,
                                              space="PSUM"))
        pso = octx.enter_context(tc.tile_pool(name="pso", bufs=2,
                                              space="PSUM"))

        # q/k rotated, per head-pair tile: rows = [hA: y1(32)|y2(32) | hB:..]
        qrot = [qkp.tile([128, S], BF16, tag=f"qrot{j}", name=f"qrot{j}")
                for j in range(2)]
        krot = [qkp.tile([128, S], BF16, tag=f"krot{j}", name=f"krot{j}")
                for j in range(2)]
        # v natural with per-head ones column: cols lh*65..lh*65+64 = v head
        # lh, col lh*65+64 = 1.0. Padded to 324 cols so the PV stationary
        # is always a 128-col slice (keeps FWL enabled).
        vsb = [vp.tile([128, 324], BF16, tag=f"v{st}", name=f"v{st}")
               for st in range(NST)]

        # ---- input DMAs ----
        # x: two big strided DMAs per chunk on the sync queue (big
        # transfers get full HBM bandwidth)
        xt_all = xtp.tile([128, NDT * S], BF16, tag="xt", name="xt")
        xt = [xt_all[:, dt * S:(dt + 1) * S] for dt in range(NDT)]
        xt_3d = xt_all.rearrange("p (d c) -> p d c", c=S)
        for sc in range(NQC):
            c0, c1 = sc * QC, (sc + 1) * QC
            for h in range(2):
                d0, d1 = h * 4, h * 4 + 4
                nc.sync.dma_start(
                    xt_3d[:, d0:d1, c0:c1],
                    xbf[sc][d0:d1].rearrange("d p c -> p d c"))

        # weights split across the gpsimd/scalar queues, ordered by need
        wq = pers.tile([128, 2048], BF16, tag="wq", name="wq")
        nc.gpsimd.dma_start(wq[:, 0:1024], wqt[:, 0:1024])
        nc.scalar.dma_start(wq[:, 1024:2048], wqt[:, 1024:2048])
        wk = pers.tile([128, 2048], BF16, tag="wk", name="wk")
        nc.gpsimd.dma_start(wk[:, 0:1024], wkt[:, 0:1024])
        nc.scalar.dma_start(wk[:, 1024:2048], wkt[:, 1024:2048])
        cos_sb = pers.tile([128, S], BF16, tag="cos")
        sin_sb = pers.tile([128, S], BF16, tag="sin")
        nc.scalar.dma_start(cos_sb[:, 0:1024], cosd[:, 0:1024])
        nc.scalar.dma_start(sin_sb[:, 0:1024], sind[:, 0:1024])
        wv = pers.tile([128, 2048], BF16, tag="wv", name="wv")
        nc.gpsimd.dma_start(wv[:], wvt)
        tri = pers.tile([128, 128], BF16, tag="tri")
        nc.scalar.dma_start(tri[:], trib)
        ident = pers.tile([128, 128], BF16, tag="ident")
        nc.scalar.dma_start(ident[:], identd)
        nc.scalar.dma_start(cos_sb[:, 1024:2048], cosd[:, 1024:2048])
        nc.scalar.dma_start(sin_sb[:, 1024:2048], sind[:, 1024:2048])
        wo_sb = []
        for hp in range(2):
            w = pers.tile([128, D], BF16, tag=f"wo{hp}", name=f"wo{hp}")
            nc.gpsimd.dma_start(w[:], wot[hp])
            wo_sb.append(w)

        # scratch operand for HAM warmup matmuls
        scr = pers.tile([128, QC], BF16, tag="scr")
        nc.vector.memset(scr[:], 0.001)
        for st in range(NST):
            vdst = vsb[st][:, 0:260].rearrange("p (h c) -> p h c",
                                               c=65)[:, :, 64:65]
            nc.vector.memset(vdst, 1.0)
            nc.vector.memset(vsb[st][:, 260:324], 0.0)

        # pre-clear the two scores PSUM buffers: the fused diagonal exp
        # reads a dead gap the matmuls never wrote
        ps_init = [pss.tile([128, 1024], F32, tag="ps", name="ps_")
                   for _ in range(2)]
        for t in ps_init:
            nc.vector.memset(t[:], 0.0)

        def warm_mm():
            wt = pgen.tile([128, QC], F32, tag="gen", name="gen")
            nc.tensor.matmul(wt[:], scr[:, 0:128], scr[:],
                             start=True, stop=True)

        for _ in range(NWARM):
            warm_mm()

        # ---- projection groups ----
        def emit_QK(sc, wi, jt, warm=False):
            c0, c1 = sc * QC, (sc + 1) * QC
            w, rot = (wq, qrot) if wi == 0 else (wk, krot)
            if warm:
                warm_mm()
            pq = pgen.tile([128, QC], F32, tag="gen", name="gen")
            for dt in range(NDT):
                nc.tensor.matmul(
                    pq[:],
                    w[:, dt * 256 + jt * 128:dt * 256 + jt * 128 + 128],
                    xt[dt][:, c0:c1],
                    start=(dt == 0), stop=(dt == NDT - 1))
            xb = s1p.tile([128, QC], BF16, tag="xb", bufs=3, name="xb")
            nc.vector.tensor_copy(xb[:], pq[:])
            t1 = s1p.tile([128, QC], BF16, tag="t1", bufs=2, name="t1")
            nc.vector.tensor_mul(t1[:], xb[:], cos_sb[:, c0:c1])
            # t2[p] = x[swap32(p)] * sneg[swap32(p)]: swap32 flips adjacent
            # 32-row blocks; sneg carries -sin on the x2 rows, so
            # rot = t1 + t2 is the full rotation. (in-operands must share
            # a base partition; only the out base may shift.)
            t2 = s1p.tile([128, QC], BF16, tag="t2", bufs=2, name="t2")
            for blk in range(4):
                a, b_ = blk * 32, (blk ^ 1) * 32
                nc.vector.tensor_mul(t2[a:a + 32, :],
                                     xb[b_:b_ + 32, :],
                                     sin_sb[b_:b_ + 32, c0:c1])
            nc.vector.tensor_add(rot[jt][:, c0:c1], t1[:], t2[:])

        def emit_V(st, warm=False):
            if warm:
                warm_mm()
            pv = pgen.tile([128, QC], F32, tag="gen", name="gen")
            for dt in range(NDT):
                nc.tensor.matmul(
                    pv[:, 0:256],
                    xt[dt][:, st * 128:(st + 1) * 128],
                    wv[:, dt * 256:(dt + 1) * 256],
                    start=(dt == 0), stop=(dt == NDT - 1))
            dst = vsb[st][:, 0:260].rearrange("p (h c) -> p h c",
                                              c=65)[:, :, 0:64]
            src = pv[:, 0:256].rearrange("p (h c) -> p h c", c=64)
            nc.scalar.copy(dst, src)

        # ---- output projection ----
        pend_y = [None]

        def emit_y(qc, otn):
            for j in range(4):
                st = 4 * qc + j
                ysb = s2p.tile([128, D], F32, tag="ysb", bufs=3,
                               name="ysb")
                for mc in range(2):
                    py = pgen.tile([128, QC], F32, tag="gen", name="gen")
                    for hp in range(2):
                        nc.tensor.matmul(
                            py[:],
                            otn[hp][:, j * 128:(j + 1) * 128],
                            wo_sb[hp][:, mc * 512:(mc + 1) * 512],
                            start=(hp == 0), stop=(hp == 1))
                    # stage the two halves on different engines (DVE is
                    # the loaded one; ACT has slack around the exps)
                    if mc == 0:
                        nc.vector.tensor_copy(ysb[:, 0:512], py[:])
                    else:
                        nc.scalar.copy(ysb[:, 512:1024], py[:])
                    q_ = nc.sync if mc == 0 else nc.gpsimd
                    q_.dma_start(
                        y[st * 128:(st + 1) * 128,
                          mc * 512:(mc + 1) * 512],
                        ysb[:, mc * 512:(mc + 1) * 512])

        # qc=3 output projection, split per head-pair so the PE isn't
        # idle-then-bursty after the last ACT-bound attention rounds
        y3sb = [s2p.tile([128, D], F32, tag="y3", bufs=4, name="y3")
                for _ in range(4)]

        def emit_y3_part0(otn0):
            for j in range(4):
                for mc in range(2):
                    py = pgen.tile([128, QC], F32, tag="gen", name="gen")
                    nc.tensor.matmul(
                        py[:], otn0[:, j * 128:(j + 1) * 128],
                        wo_sb[0][:, mc * 512:(mc + 1) * 512],
                        start=True, stop=True)
                    nc.vector.tensor_copy(
                        y3sb[j][:, mc * 512:(mc + 1) * 512], py[:])

        def emit_y3_final(otn1):
            for j in range(4):
                st = 12 + j
                for mc in range(2):
                    py = pgen.tile([128, QC], F32, tag="gen", name="gen")
                    nc.tensor.matmul(
                        py[:], otn1[:, j * 128:(j + 1) * 128],
                        wo_sb[1][:, mc * 512:(mc + 1) * 512],
                        start=True, stop=True)
                    nc.vector.tensor_add(
                        y3sb[j][:, mc * 512:(mc + 1) * 512],
                        y3sb[j][:, mc * 512:(mc + 1) * 512], py[:])
                    q_ = nc.sync if mc == 0 else nc.gpsimd
                    q_.dma_start(
                        y[st * 128:(st + 1) * 128,
                          mc * 512:(mc + 1) * 512],
                        y3sb[j][:, mc * 512:(mc + 1) * 512])

        # ---- attention window for one q-chunk ----
        def emit_att(qc, fillers=()):
            fillers = list(fillers)
            fi = [0]

            def take_filler():
                if fi[0] < len(fillers):
                    fillers[fi[0]]()
                    fi[0] += 1

            npair = 2 * qc + 2
            otn = [s2p.tile([128, QC], BF16, tag=f"otn{hp}", bufs=2,
                            name=f"otn{hp}") for hp in range(2)]
            for hp in range(2):          # head pair = (2hp, 2hp+1)
                po = [pso.tile([128, QC], F32, tag="po", name="po")
                      for _ in range(2)]

                def emit_pv(kp, pts, mlist, last, hp=hp, po=po):
                    for z in range(2):
                        lh = 2 * hp + z
                        for e in range(2):
                            kb = 2 * kp + e
                            m = mlist[e]
                            nc.tensor.matmul(
                                po[z][0:128, m:QC],
                                vsb[kb][:, lh * 65:lh * 65 + 128],
                                pts[z][:, e * QC + m:(e + 1) * QC],
                                start=(kb == 0),
                                stop=(last and e == 1))

                pend = []
                for ki in range(npair):
                    kp = ki
                    diag = kp >= npair - 2
                    mlist = []
                    for e in range(2):
                        kb = 2 * kp + e
                        r = kb - 4 * qc
                        mlist.append(128 * r if diag else 0)
                    ps_pair = [pss.tile([128, 1024], F32, tag="ps",
                                        name="ps_") for _ in range(2)]
                    # e-outer / z-inner: the two z matmuls hit disjoint
                    # PE row groups and are adjacent in issue order, so
                    # they co-stream (z-outer order head-of-line blocks
                    # on the same-row-group e sibling and serializes)
                    for e in range(2):
                        kb = 2 * kp + e
                        m = mlist[e]
                        for z in range(2):
                            r0 = 64 * z
                            nc.tensor.matmul(
                                ps_pair[z][:, e * QC + m:(e + 1) * QC],
                                krot[hp][r0:r0 + 64,
                                         kb * 128:(kb + 1) * 128],
                                qrot[hp][r0:r0 + 64,
                                         qc * QC + m:(qc + 1) * QC],
                                start=True, stop=not diag,
                                tile_position=(r0, 0))
                    if diag:
                        # causal triangle as a PE-side additive bias
                        # (-240 above the diagonal) on the first 128
                        # alive columns
                        for e in range(2):
                            m = mlist[e]
                            for z in range(2):
                                nc.tensor.matmul(
                                    ps_pair[z][:,
                                               e * QC + m:e * QC + m + 128],
                                    tri[:], ident[:],
                                    start=False, stop=True,
                                    skip_group_check=True)
                    pts = []
                    for z in range(2):
                        pt = s2p.tile([128, 1024], BF16, tag="pt",
                                      bufs=6, name="pt")
                        if not diag:
                            nc.scalar.activation(pt[:], ps_pair[z][:],
                                                 Exp, scale=0.125)
                        elif mlist[0] == 0:
                            # first diagonal round: one exp across both
                            # e-blocks; the dead gap cols hold bounded
                            # stale PSUM and are never read by PV
                            nc.scalar.activation(pt[:], ps_pair[z][:],
                                                 Exp, scale=0.125)
                        else:
                            for e in range(2):
                                m = mlist[e]
                                nc.scalar.activation(
                                    pt[:, e * QC + m:(e + 1) * QC],
                                    ps_pair[z][:, e * QC + m:(e + 1) * QC],
                                    Exp, scale=0.125)
                        pts.append(pt)
                    # software-pipeline: PV lags two rounds behind the
                    # scores so the PE never waits on exp latency
                    pend.append((kp, pts, mlist, ki == npair - 1))
                    if len(pend) > 2:
                        emit_pv(*pend.pop(0))
                    if ki == 1 and hp == 1 and pend_y[0] is not None:
                        emit_y(*pend_y[0])
                        pend_y[0] = None
                    if qc == 3 and hp == 1 and ki == 4:
                        emit_y3_part0(otn[0])
                    if ki < npair - 1:
                        take_filler()
                for args in pend:
                    emit_pv(*args)

                # stage O^T+den out of PSUM in one copy per z (frees po
                # for the next head pair), then 1/den -> broadcast ->
                # normalize; broadcast+multiply live on GpSimd
                otu2 = []
                dsb2 = []
                for z in range(2):
                    ot = s2p.tile([64, QC], F32, tag="otu", bufs=4,
                                  name="otu")
                    nc.vector.tensor_copy(ot[:], po[z][0:64, :])
                    otu2.append(ot)
                    ds = s2p.tile([1, QC], F32, tag="dsb", bufs=4,
                                  name="dsb")
                    nc.vector.tensor_copy(ds[:], po[z][64:65, :])
                    dsb2.append(ds)
                for z in range(2):
                    r = s2p.tile([1, QC], F32, tag="rsb", bufs=4,
                                 name="rsb")
                    nc.vector.reciprocal_approx_fast(
                        out=r[:], in_=dsb2[z][:])
                    pbs = s2p.tile([64, QC], F32, tag="pbs", bufs=4,
                                   name="pbs")
                    nc.gpsimd.partition_broadcast(pbs[:], r[:])
                    nc.vector.tensor_mul(otn[hp][64 * z:64 * z + 64, :],
                                         otu2[z][:], pbs[:])
            pend_y[0] = (qc, otn)
            # leftover fillers (window had fewer slots than fillers)
            while fi[0] < len(fillers):
                fillers[fi[0]]()
                fi[0] += 1

        # ---- schedule: interleave projection chunks between the ----
        # ---- ACT-bound attention windows; dependency-free warm  ----
        # ---- matmuls keep HAM at full clock through the         ----
        # ---- exp-bound rounds                                   ----
        def emit_P(sc, warm=False):
            for wi in range(2):
                for jt in range(2):
                    emit_QK(sc, wi, jt, warm=warm)
            for st in range(4 * sc, 4 * sc + 4):
                emit_V(st, warm=warm)

        emit_P(0, warm=True)
        emit_P(1, warm=True)
        emit_att(0)
        emit_P(2)
        emit_att(1)
        emit_P(3)
        emit_att(2, fillers=[warm_mm] * 10)
        emit_att(3, fillers=[warm_mm] * 12)
        emit_y3_final(pend_y[0][1][1])

    nc.compile()
    return nc


def _get_nc():
    global _NC_CACHE
    if _NC_CACHE is None:
        _NC_CACHE = _build()
    return _NC_CACHE


def _host_prep(x, Wq, Wk, Wv, Wo):
    """Build the 8 per-core input maps."""
    bf = ml_dtypes.bfloat16
    x = np.asarray(x, dtype=np.float32)
    Wq, Wk, Wv, Wo = (np.asarray(w, dtype=np.float32) for w in (Wq, Wk, Wv, Wo))

    def tile128(wt):  # [1024, 256] -> [128, 2048] with d-tiles along free dim
        return np.ascontiguousarray(
            wt.reshape(NDT, 128, LD).transpose(1, 0, 2).reshape(
                128, NDT * LD).astype(bf))

    def perm_qk(W, g):
        # rows per jt tile: [h0 evens(32) | h0 odds(32) | h1 evens | h1 odds]
        blocks = []
        for lh in range(LH):
            gh = g * LH + lh
            O = W[gh * HD:(gh + 1) * HD]          # [64, 1024]
            blocks.append(O[0::2])
            blocks.append(O[1::2])
        Wp = np.concatenate(blocks, axis=0)       # [256, 1024]
        return tile128(Wp.T)

    t = np.arange(32, dtype=np.float64)
    theta = 1.0 / (ROPE_BASE ** (2.0 * t / HD))
    ang = np.arange(S, dtype=np.float64)[:, None] * theta[None, :]  # [S, 32]
    c32 = np.cos(ang).T.astype(np.float32)        # [32, S]
    s32 = np.sin(ang).T.astype(np.float32)
    cosd = np.ascontiguousarray(np.tile(c32, (4, 1)).astype(bf))  # [128, S]
    sneg = np.tile(s32, (4, 1))
    sneg[32:64] *= -1.0   # x2 rows carry -sin so rot = x*cos + swap(x*sneg)
    sneg[96:128] *= -1.0
    sind = np.ascontiguousarray(sneg.astype(bf))

    k_ = np.arange(128)[:, None]
    p_ = np.arange(128)[None, :]
    # additive causal bias, laid out [k, p]: out[p, j] += trib[j, p]
    trib = np.ascontiguousarray((-240.0 * (p_ > k_)).astype(bf))
    identd = np.ascontiguousarray(np.eye(128).astype(bf))

    per_b = []
    for b in range(B):
        # pre-transposed, chunked [sc, dt, 128 (d), QC (s)]
        xb = x[b].astype(bf)                       # [S, D]
        xb = xb.reshape(NQC, QC, NDT, 128).transpose(0, 2, 3, 1)
        per_b.append(np.ascontiguousarray(xb))

    per_g = []
    for g in range(TPG):
        wq = perm_qk(Wq, g)
        wk = perm_qk(Wk, g)
        wvt_ = tile128(Wv[g * LD:(g + 1) * LD].T)
        wot_ = np.ascontiguousarray(
            Wo[:, g * LD:(g + 1) * LD].T.reshape(2, 128, D).astype(bf))
        per_g.append((wq, wk, wvt_, wot_))

    in_maps = []
    for core in range(NCORES):
        b, g = divmod(core, TPG)
        wq, wk, wvt_, wot_ = per_g[g]
        in_maps.append({
            "xbf": per_b[b],
            "wqt": wq, "wkt": wk, "wvt": wvt_, "wot": wot_,
            "cosd": cosd, "sind": sind, "trib": trib, "identd": identd,
        })
    return in_maps


def run(inputs, trace=False):
    """Run on all 8 cores; returns (y_full, BassKernelResults)."""
    x = inputs["x"]
    in_maps = _host_prep(x, inputs["Wq"], inputs["Wk"], inputs["Wv"],
                         inputs["Wo"])
    nc = _get_nc()
    kw = {}
    if trace:
        kw = dict(trace=True, trace_cores=[0])
    res = run_bass_kernel_spmd(nc, in_maps, core_ids=list(range(NCORES)), **kw)
    y = np.zeros((B, S, D), dtype=np.float32)
    for c in range(NCORES):
        y[c // TPG] += res.results[c]["y"]
    return y, res


def kernel(x, Wq, Wk, Wv, Wo, n_heads):
    assert int(n_heads) == H
    y, _ = run({"x": x, "Wq": Wq, "Wk": Wk, "Wv": Wv, "Wo": Wo})
    return y
